# revision 7
# baseline (speedup 1.0000x reference)
"""dma_gather-based edge dot-product kernel for 8 trn2 NeuronCores.

score[e] = <h_src[src_idx[e]], h_dst[dst_idx[e]]>, E=625000, D=128, N=100000.

Edges are sharded across the 8 cores (78125 each). dma_gather needs int16
indices, so the host bins each core's edges into 16 buckets by
(src_idx//25000, dst_idx//25000); in-bucket indices fit int16 against the
matching table quarter. Each bucket is one dma_gather pair (thousands of
512B rows per instruction, amortizing the ~1us SWDGE setup ~40x vs
indirect_dma_start's 128 rows/call). Gathered rows land at
dst[i%128, i//128, :]; DVE multiply + grouped reduce produce the scores and
the host un-permutes them with the binning order. Oversized buckets (only
possible for pathologically skewed indices) are split into chunks so SBUF
tiles stay bounded.
"""

import numpy as np

N = 100000
D = 128
E = 625000
N_CORES = 8
E_CORE = E // N_CORES          # 78125
P = 128
Q = N // 4                     # 25000 rows per table quarter
CMAX = 12800                   # per-chunk edge cap (SBUF: 2 tags x 2 bufs x 50KB)

_cache = {}


def _build_bass(C, chunk_map, reps=1):
    """C = per-chunk edge capacity (mult of 128); chunk_map = ((qs, qd), ...).

    reps > 1 unrolls the whole pipeline for steady-state timing benches.
    """
    import concourse.bacc as bacc
    import concourse.tile as tile
    from concourse import mybir

    NCH = len(chunk_map)
    CB = C // 128              # 512B chunks per partition per gather
    CW = C // 16               # idx words per partition (16-way wrap)
    # s+t tags each get `bufs` slots of C*4 bytes/partition; keep the big
    # pool under ~160KB/partition of the 208KB usable.
    bufs = max(2, min(4, (160 * 1024) // (C * 4 * 2)))

    nc = bacc.Bacc("TRN2", target_bir_lowering=False)

    h_src_q = [
        nc.dram_tensor(f"h_src{q}", [Q, D], mybir.dt.float32, kind="ExternalInput")
        for q in range(4)
    ]
    h_dst_q = [
        nc.dram_tensor(f"h_dst{q}", [Q, D], mybir.dt.float32, kind="ExternalInput")
        for q in range(4)
    ]
    src_g = nc.dram_tensor("src_g", [NCH, P, CW], mybir.dt.int16, kind="ExternalInput")
    dst_g = nc.dram_tensor("dst_g", [NCH, P, CW], mybir.dt.int16, kind="ExternalInput")
    scores = nc.dram_tensor("scores", [P, NCH * CB], mybir.dt.float32,
                            kind="ExternalOutput")

    with tile.TileContext(nc) as tc:
        with (
            tc.tile_pool(name="big", bufs=bufs) as big,
            tc.tile_pool(name="idx", bufs=3) as idxp,
            tc.tile_pool(name="acc", bufs=min(2, reps)) as acc,
        ):
          for _rep in range(reps):
            sc_full = acc.tile([P, NCH * CB], mybir.dt.float32, tag="sc")
            for b, (qs, qd) in enumerate(chunk_map):
                i_s = idxp.tile([P, CW], mybir.dt.int16, tag="i_s")
                i_d = idxp.tile([P, CW], mybir.dt.int16, tag="i_d")
                nc.sync.dma_start(out=i_s[:], in_=src_g[b])
                nc.sync.dma_start(out=i_d[:], in_=dst_g[b])

                s_tile = big.tile([P, C], mybir.dt.float32, tag="s")
                t_tile = big.tile([P, C], mybir.dt.float32, tag="t")
                nc.gpsimd.dma_gather(
                    out_ap=s_tile[:].rearrange("p (c d) -> p c d", d=D),
                    in_ap=h_src_q[qs][:],
                    idxs_ap=i_s[:],
                    num_idxs=C,
                    num_idxs_reg=C,
                    elem_size=D,
                    single_packet=False,
                )
                nc.gpsimd.dma_gather(
                    out_ap=t_tile[:].rearrange("p (c d) -> p c d", d=D),
                    in_ap=h_dst_q[qd][:],
                    idxs_ap=i_d[:],
                    num_idxs=C,
                    num_idxs_reg=C,
                    elem_size=D,
                    single_packet=False,
                )
                nc.vector.tensor_tensor(
                    out=s_tile[:], in0=s_tile[:], in1=t_tile[:],
                    op=mybir.AluOpType.mult,
                )
                nc.vector.tensor_reduce(
                    out=sc_full[:, b * CB:(b + 1) * CB],
                    in_=s_tile[:].rearrange("p (c d) -> p c d", d=D),
                    axis=mybir.AxisListType.X,
                    op=mybir.AluOpType.add,
                )
            nc.sync.dma_start(out=scores[:, :], in_=sc_full[:])
    nc.finalize()
    return nc


def _prepare_reps(h_src, h_dst, src_idx, dst_idx, reps):
    """Like _prepare but builds an uncached reps-unrolled program (bench only)."""
    global _REPS_OVERRIDE
    _REPS_OVERRIDE = reps
    try:
        return _prepare(h_src, h_dst, src_idx, dst_idx)
    finally:
        _REPS_OVERRIDE = 1


_REPS_OVERRIDE = 1


def _wrap16(vals, C):
    """int16 stream [cnt] -> [P, C//16] tile (16-way wrap, replicated x8)."""
    w = np.zeros(C, dtype=np.int16)
    w[:len(vals)] = vals
    w16 = w.reshape(C // 16, 16).T          # [16, C//16]
    return np.tile(w16, (8, 1))             # [128, C//16]


LAST_RESULTS = None


def _prepare(h_src, h_dst, src_idx, dst_idx):
    """Host-side prep: bin edges, build idx tiles, compile Bass program.

    Returns (nc, in_maps, meta) where meta carries what _postprocess needs.
    """
    h_src = np.ascontiguousarray(np.asarray(h_src, dtype=np.float32))
    h_dst = np.ascontiguousarray(np.asarray(h_dst, dtype=np.float32))
    src_idx = np.asarray(src_idx).astype(np.int64)
    dst_idx = np.asarray(dst_idx).astype(np.int64)

    # --- host binning: per core, group edges by (src quarter, dst quarter) ---
    orders, counts_all = [], []
    for c in range(N_CORES):
        ss = src_idx[c * E_CORE:(c + 1) * E_CORE]
        dd = dst_idx[c * E_CORE:(c + 1) * E_CORE]
        bins = (ss // Q) * 4 + (dd // Q)
        orders.append(np.argsort(bins, kind="stable"))
        counts_all.append(np.bincount(bins, minlength=16))

    # Split oversized bins into chunks of <= CMAX edges. All cores share one
    # chunk layout (same compiled program); chunk b of bin (qs,qd) holds that
    # bin's edges [b*C, b*C+cnt) in stream order.
    max_cnt = max(int(cc.max()) for cc in counts_all)
    n_split = -(-max_cnt // CMAX)                       # chunks per bin
    per_chunk_max = -(-max_cnt // n_split)
    C = max(512, -(-per_chunk_max // 128) * 128)
    CB = C // 128
    chunk_map = tuple(
        (qs, qd) for qs in range(4) for qd in range(4) for _ in range(n_split)
    )
    NCH = len(chunk_map)

    key = (C, chunk_map, _REPS_OVERRIDE)
    if key not in _cache:
        _cache[key] = _build_bass(C, chunk_map, reps=_REPS_OVERRIDE)
    nc = _cache[key]

    def chunk_counts(counts):
        """Per-chunk edge counts for one core, matching chunk_map order."""
        out = []
        for b in range(16):
            left = int(counts[b])
            for _ in range(n_split):
                take = min(left, C)
                out.append(take)
                left -= take
        return out

    in_maps = []
    for c in range(N_CORES):
        ss = src_idx[c * E_CORE:(c + 1) * E_CORE]
        dd = dst_idx[c * E_CORE:(c + 1) * E_CORE]
        order = orders[c]
        sso, ddo = ss[order], dd[order]
        src_b = np.empty((NCH, P, C // 16), dtype=np.int16)
        dst_b = np.empty((NCH, P, C // 16), dtype=np.int16)
        off = 0
        for ch, ((qs, qd), cnt) in enumerate(zip(chunk_map, chunk_counts(counts_all[c]))):
            src_b[ch] = _wrap16((sso[off:off + cnt] - qs * Q).astype(np.int16), C)
            dst_b[ch] = _wrap16((ddo[off:off + cnt] - qd * Q).astype(np.int16), C)
            off += cnt
        im = {"src_g": src_b, "dst_g": dst_b}
        for q in range(4):
            im[f"h_src{q}"] = np.ascontiguousarray(h_src[q * Q:(q + 1) * Q])
            im[f"h_dst{q}"] = np.ascontiguousarray(h_dst[q * Q:(q + 1) * Q])
        in_maps.append(im)

    meta = {
        "orders": orders,
        "counts_all": counts_all,
        "CB": CB,
        "chunk_counts": chunk_counts,
    }
    return nc, in_maps, meta


def _postprocess(results, meta):
    orders = meta["orders"]
    counts_all = meta["counts_all"]
    CB = meta["CB"]
    chunk_counts = meta["chunk_counts"]
    out = np.empty(E, dtype=np.float32)
    for c in range(N_CORES):
        sc = results[c]["scores"]            # [P, NCH*CB]
        perm_scores = np.empty(E_CORE, dtype=np.float32)
        off = 0
        for ch, cnt in enumerate(chunk_counts(counts_all[c])):
            i = np.arange(cnt)
            perm_scores[off:off + cnt] = sc[i % P, ch * CB + i // P]
            off += cnt
        core_scores = np.empty(E_CORE, dtype=np.float32)
        core_scores[orders[c]] = perm_scores
        out[c * E_CORE:(c + 1) * E_CORE] = core_scores
    return out.reshape(E, 1)


def kernel(h_src, h_dst, src_idx, dst_idx):
    global LAST_RESULTS
    from concourse.bass_utils import run_bass_kernel_spmd

    nc, in_maps, meta = _prepare(h_src, h_dst, src_idx, dst_idx)
    res = run_bass_kernel_spmd(nc, in_maps, core_ids=list(range(N_CORES)))
    LAST_RESULTS = res
    return _postprocess(res.results, meta)

